# revision 1
# baseline (speedup 1.0000x reference)
import sys

import numpy as np

try:
    from concourse import bacc, bass, tile, masks
    from concourse.bass_utils import run_bass_kernel_spmd
except ImportError:
    sys.path.insert(0, "/opt/trn_rl_repo")
    from concourse import bacc, bass, tile, masks
    from concourse.bass_utils import run_bass_kernel_spmd

mybir = bass.mybir

N, D, F, H = 8192, 256, 256, 256
NC = 8
RPC = N // NC           # rows per core
TILES = RPC // 128      # 128-row tiles per core
LN_EPS = 1e-5
DENOM_EPS = 1e-8
FP = mybir.dt.float32
FPR = mybir.dt.float32r
AF = mybir.ActivationFunctionType
ALU = mybir.AluOpType
AX = mybir.AxisListType
SA = H + 2  # augmented cols padded even for fp32r ISA restriction

R_PROJ = False  # fp32r for q/k/v projection matmuls
R_REST = False  # fp32r for S / T_aug / num / ffn matmuls


def _mm(nc, out, lhsT, rhs, start, stop, fast=False):
    nc.tensor.matmul(out, lhsT, rhs, start=start, stop=stop)


def _layer_norm(nc, pool, out_ap, in_ap, eps_ap):
    stats = pool.tile([128, 6], FP)
    aggr = pool.tile([128, 2], FP)
    std = pool.tile([128, 1], FP)
    rstd = pool.tile([128, 1], FP)
    nc.vector.bn_stats(stats[:], in_ap)
    nc.vector.bn_aggr(aggr[:], stats[:])
    nc.scalar.activation(std[:], aggr[:, 1:2], AF.Sqrt, bias=eps_ap)
    nc.vector.reciprocal(rstd[:], std[:])
    nc.vector.tensor_scalar(
        out_ap, in_ap, aggr[:, 0:1], rstd[:], ALU.subtract, ALU.mult
    )


def _build_kernel():
    nc = bacc.Bacc(None, target_bir_lowering=False)
    PDT = FPR if R_PROJ else FP
    RDT = FPR if R_REST else FP

    x_in = nc.declare_dram_parameter("x", [RPC, D], FP, isOutput=False)
    wqr_in = nc.declare_dram_parameter("wqr", [D, F], PDT, isOutput=False)
    wkr_in = nc.declare_dram_parameter("wkr", [D, F], PDT, isOutput=False)
    wv_in = nc.declare_dram_parameter("wv", [D, H], PDT, isOutput=False)
    w1_in = nc.declare_dram_parameter("w1", [D, H], RDT, isOutput=False)
    w2_in = nc.declare_dram_parameter("w2", [H, D], RDT, isOutput=False)
    out_ext = nc.declare_dram_parameter("out", [RPC, D], FP, isOutput=True)

    with tile.TileContext(nc) as tc:
        with (
            tc.tile_pool(name="const", bufs=1) as const_pool,
            tc.tile_pool(name="wpool", bufs=1) as wpool,
            tc.tile_pool(name="store", bufs=1) as store_pool,
            tc.tile_pool(name="dram", bufs=1, space="DRAM") as dram_pool,
        ):
            ident = const_pool.tile([128, 128], FP)
            masks.make_identity(nc, ident[:])
            eps_t = const_pool.tile([128, 1], FP)
            nc.vector.memset(eps_t[:], LN_EPS)
            ones_t = const_pool.tile([128, 2], FP)
            nc.vector.memset(ones_t[:], 1.0)

            def load_w(dram_t, name):
                t = wpool.tile([128, 2, 256], dram_t.dtype, name=name)
                for c in (0, 1):
                    nc.sync.dma_start(
                        out=t[:, c, :], in_=dram_t[c * 128 : (c + 1) * 128, :]
                    )
                return t

            # k-side weights first: they gate phase A1 start.
            wkr = load_w(wkr_in, "wkr_sb")
            wv = load_w(wv_in, "wv_sb")

            x_store = store_pool.tile([128, TILES, D], FP)
            for t in range(TILES):
                nc.sync.dma_start(
                    out=x_store[:, t, :], in_=x_in[t * 128 : (t + 1) * 128, :]
                )

            wqr = load_w(wqr_in, "wqr_sb")
            w1 = load_w(w1_in, "w1_sb")
            w2 = load_w(w2_in, "w2_sb")

            xT_store = store_pool.tile([128, TILES, 2, 128], PDT)
            eqT_store = store_pool.tile([128, TILES, 2, 128], RDT)
            s_sb = store_pool.tile([128, 2, SA], FP)
            # Wo is folded into Wv host-side (wv = Wv@Wo), so the
            # AllReduduced S is directly T_aug = [S_v@Wo | colsum].
            s_red = store_pool.tile([128, 2, SA], FP)

            # ---------------- Phase A1: k-side -> local S ---------------------
            with (
                tc.tile_pool(name="a_sb", bufs=3) as a_sb,
                tc.tile_pool(name="a_ps", bufs=2, space="PSUM") as a_ps,
                tc.tile_pool(name="s_ps", bufs=1, space="PSUM") as s_ps,
            ):
                s_psum = [
                    s_ps.tile([128, SA], FP, name=f"s_psum{c}") for c in (0, 1)
                ]

                ek_t = [None] * TILES
                va_t = [None] * TILES

                def a1_front(t):
                    xt_ps = a_ps.tile([128, 512], FP)
                    for c in (0, 1):
                        nc.tensor.transpose(
                            xt_ps[:, c * 128 : (c + 1) * 128],
                            x_store[:, t, c * 128 : (c + 1) * 128],
                            ident[:],
                        )
                        nc.vector.tensor_scalar_add(
                            xT_store[:, t, c, :],
                            xt_ps[:, c * 128 : (c + 1) * 128],
                            0.0,
                        )

                    # k cols 0:256, v cols 256:512 — ONE accumulation group
                    # (start zeroes the whole 2KB PSUM bank).
                    kv_ps = a_ps.tile([128, 512], FP)
                    for c in (0, 1):
                        _mm(nc, kv_ps[:, 0:256], xT_store[:, t, c, :], wkr[:, c, :],
                            c == 0, False, R_PROJ)
                        _mm(nc, kv_ps[:, 256:512], xT_store[:, t, c, :], wv[:, c, :],
                            False, c == 1, R_PROJ)

                    nmk = a_sb.tile([128, 1], FP)
                    nc.vector.tensor_reduce(
                        out=nmk[:], in_=kv_ps[:, 0:256], axis=AX.X, op=ALU.max,
                        negate=True,
                    )
                    ek = a_sb.tile([128, F], RDT, name="ek_keep")
                    nc.scalar.activation(ek[:], kv_ps[:, 0:256], AF.Exp, bias=nmk[:])

                    v_aug = a_sb.tile([128, SA], RDT, name="va_keep")
                    nc.scalar.copy(v_aug[:, 0:H], kv_ps[:, 256:512])
                    nc.scalar.copy(v_aug[:, H:SA], ones_t[:])
                    ek_t[t] = ek
                    va_t[t] = v_aug

                def a1_smm(t):
                    for c in (0, 1):
                        _mm(nc, s_psum[c][:], ek_t[t][:, c * 128 : (c + 1) * 128],
                            va_t[t][:], t == 0, t == TILES - 1, R_REST)

                # one-tile skew: S matmuls never stall the tensor queue on exp
                for t in range(TILES):
                    a1_front(t)
                    if t >= 1:
                        a1_smm(t - 1)
                a1_smm(TILES - 1)

                for c in (0, 1):
                    nc.scalar.copy(s_sb[:, c, :], s_psum[c][:])

            # ---------------- AllReduce of S_aug across 8 cores --------------
            cc_in = dram_pool.tile([128, 2, SA], FP)
            cc_out = dram_pool.tile([128, 2, SA], FP, addr_space="Shared")
            nc.sync.dma_start(out=cc_in[:], in_=s_sb[:])
            nc.gpsimd.collective_compute(
                "AllReduce",
                ALU.add,
                replica_groups=[list(range(NC))],
                ins=[cc_in[:].opt()],
                outs=[cc_out[:].opt()],
            )
            for c in (0, 1):
                nc.sync.dma_start(out=s_red[:, c, :], in_=cc_out[:, c, :])

            # ---------------- Phase A2: q-side (runs under the AllReduce) ----
            with (
                tc.tile_pool(name="q_sb", bufs=3) as q_sb,
                tc.tile_pool(name="q_ps", bufs=2, space="PSUM") as q_ps,
            ):
                for t in range(TILES):
                    qp_ps = q_ps.tile([128, 512], FP)
                    for c in (0, 1):
                        _mm(nc, qp_ps[:, 0:256], xT_store[:, t, c, :], wqr[:, c, :],
                            c == 0, c == 1, R_PROJ)
                    nmq = q_sb.tile([128, 1], FP)
                    nc.vector.tensor_reduce(
                        out=nmq[:], in_=qp_ps[:, 0:256], axis=AX.X, op=ALU.max,
                        negate=True,
                    )
                    eq = q_sb.tile([128, F], FP)
                    nc.scalar.activation(eq[:], qp_ps[:, 0:256], AF.Exp, bias=nmq[:])

                    et_ps = q_ps.tile([128, 512], FP)
                    for c in (0, 1):
                        nc.tensor.transpose(
                            et_ps[:, c * 128 : (c + 1) * 128],
                            eq[:, c * 128 : (c + 1) * 128],
                            ident[:],
                        )
                        nc.scalar.copy(
                            eqT_store[:, t, c, :], et_ps[:, c * 128 : (c + 1) * 128]
                        )

            # ---------------- Phase B: numer, LN1, FFN, LN2 ------------------
            # Software-pipelined skew over tiles/pairs: engines have in-order
            # queues, so interleave independent work to avoid stalls.  ffn1 is
            # computed directly transposed (pre1T = w1_chunk.T @ hT-pair) so
            # relu lands on f1T and no f1 transposes are needed.
            NPAIR = TILES // 2
            with (
                tc.tile_pool(name="b_sb", bufs=6) as b_sb,
                tc.tile_pool(name="p_num", bufs=2, space="PSUM") as p_num,
                tc.tile_pool(name="p_hT", bufs=2, space="PSUM") as p_hT,
                tc.tile_pool(name="p_ff1", bufs=2, space="PSUM") as p_ff1,
                tc.tile_pool(name="p_ff2", bufs=2, space="PSUM") as p_ff2,
            ):
                h_t = [None] * TILES
                hT_p = [None] * NPAIR
                f1T_p = [None] * NPAIR

                def stage1(t):
                    num_ps = p_num.tile([128, SA], FP, name="num_ps")
                    for c in (0, 1):
                        _mm(nc, num_ps[:], eqT_store[:, t, c, :], s_red[:, c, :],
                            c == 0, c == 1, R_REST)
                    d_sb = b_sb.tile([128, 1], FP)
                    r = b_sb.tile([128, 1], FP)
                    nc.vector.tensor_scalar_add(
                        d_sb[:], num_ps[:, H : H + 1], DENOM_EPS
                    )
                    nc.vector.reciprocal(r[:], d_sb[:])
                    hin = b_sb.tile([128, D], FP)
                    nc.vector.scalar_tensor_tensor(
                        out=hin[:],
                        in0=num_ps[:, 0:D],
                        scalar=r[:],
                        in1=x_store[:, t, :],
                        op0=ALU.mult,
                        op1=ALU.add,
                    )
                    h = b_sb.tile([128, D], FP, name="h_keep")
                    _layer_norm(nc, b_sb, h[:], hin[:], eps_t[:])
                    h_t[t] = h

                def s2_transpose(p):
                    hT_ps = p_hT.tile([128, 512], FP, name="hT_ps")
                    hT2 = b_sb.tile([128, 2, 256], RDT, name="hT2")
                    for j in (0, 1):
                        for c in (0, 1):
                            k = 2 * j + c
                            nc.tensor.transpose(
                                hT_ps[:, k * 128 : (k + 1) * 128],
                                h_t[2 * p + j][:, c * 128 : (c + 1) * 128],
                                ident[:],
                            )
                            nc.scalar.copy(
                                hT2[:, c, j * 128 : (j + 1) * 128],
                                hT_ps[:, k * 128 : (k + 1) * 128],
                            )
                    hT_p[p] = hT2

                def s2_ffn1(p):
                    pre1T = p_ff1.tile([128, 512], FP, name="pre1T")
                    hT2 = hT_p[p]
                    # one merged accumulation group (start zeroes whole bank)
                    first = True
                    for fc in (0, 1):
                        for m in (0, 1):
                            _mm(nc, pre1T[:, m * 256 : (m + 1) * 256],
                                w1[:, fc, m * 128 : (m + 1) * 128],
                                hT2[:, fc, :],
                                first, fc == 1 and m == 1, R_REST)
                            first = False
                    f1T = b_sb.tile([128, 2, 256], RDT, name="f1T2")
                    for m in (0, 1):
                        nc.scalar.activation(
                            f1T[:, m, :], pre1T[:, m * 256 : (m + 1) * 256], AF.Relu
                        )
                    f1T_p[p] = f1T

                def stage3(t):
                    p, j = t // 2, t % 2
                    f1T = f1T_p[p]
                    ff2_ps = p_ff2.tile([128, D], FP, name="ff2_ps")
                    for m in (0, 1):
                        _mm(nc, ff2_ps[:], f1T[:, m, j * 128 : (j + 1) * 128],
                            w2[:, m, :], m == 0, m == 1, R_REST)
                    h2 = b_sb.tile([128, D], FP)
                    nc.vector.scalar_tensor_tensor(
                        out=h2[:], in0=ff2_ps[:], scalar=0.0, in1=h_t[t][:],
                        op0=ALU.bypass, op1=ALU.add,
                    )
                    outt = b_sb.tile([128, D], FP)
                    _layer_norm(nc, b_sb, outt[:], h2[:], eps_t[:])
                    nc.sync.dma_start(
                        out=out_ext[t * 128 : (t + 1) * 128, :], in_=outt[:]
                    )

                for i in range(NPAIR + 2):
                    if i < NPAIR:
                        stage1(2 * i)
                        stage1(2 * i + 1)
                    if 1 <= i <= NPAIR:
                        s2_transpose(i - 1)
                    if i >= 2:
                        stage3(2 * (i - 2))
                        stage3(2 * (i - 2) + 1)
                    if 1 <= i <= NPAIR:
                        s2_ffn1(i - 1)

    nc.finalize()
    return nc


_NC_CACHE = {}


def _get_nc():
    key = (R_PROJ, R_REST)
    if key not in _NC_CACHE:
        _NC_CACHE[key] = _build_kernel()
    return _NC_CACHE[key]


def _run(inputs, trace=False, **kw):
    x = np.ascontiguousarray(inputs["x"], dtype=np.float32)
    R = inputs["R"].astype(np.float64)
    wqr = (inputs["Wq"].astype(np.float64) @ R).astype(np.float32)
    wkr = (inputs["Wk"].astype(np.float64) @ R).astype(np.float32)
    wvo = (
        inputs["Wv"].astype(np.float64) @ inputs["Wo"].astype(np.float64)
    ).astype(np.float32)
    shared = {
        "wqr": np.ascontiguousarray(wqr),
        "wkr": np.ascontiguousarray(wkr),
        "wv": np.ascontiguousarray(wvo),
        "w1": np.ascontiguousarray(inputs["W1"], dtype=np.float32),
        "w2": np.ascontiguousarray(inputs["W2"], dtype=np.float32),
    }
    in_maps = [
        {"x": np.ascontiguousarray(x[c * RPC : (c + 1) * RPC]), **shared}
        for c in range(NC)
    ]
    nc = _get_nc()
    res = run_bass_kernel_spmd(nc, in_maps, list(range(NC)), trace=trace, **kw)
    out = np.concatenate([res.results[c]["out"] for c in range(NC)], axis=0)
    return out.astype(np.float32), res


def kernel(**inputs) -> np.ndarray:
    out, _ = _run(inputs)
    return out



# revision 7
# speedup vs baseline: 1.1813x; 1.1813x over previous
import sys

import numpy as np

try:
    from concourse import bacc, bass, tile, masks
    from concourse.bass_utils import run_bass_kernel_spmd
except ImportError:
    sys.path.insert(0, "/opt/trn_rl_repo")
    from concourse import bacc, bass, tile, masks
    from concourse.bass_utils import run_bass_kernel_spmd

import ml_dtypes

mybir = bass.mybir

N, D, F, H = 8192, 256, 256, 256
NC = 8
RPC = N // NC           # rows per core
TILES = RPC // 128      # 128-row tiles per core
LN_EPS = 1e-5
QSHIFT = 64.0           # constant exp shift on q'; cancels in num/denom
FP = mybir.dt.float32
FPR = mybir.dt.float32r
BF = mybir.dt.bfloat16
AF = mybir.ActivationFunctionType
ALU = mybir.AluOpType
AX = mybir.AxisListType
SA = H + 2  # v columns + ones (denominator) + pad


def _build_kernel():
    nc = bacc.Bacc(None, target_bir_lowering=False)

    x_in = nc.declare_dram_parameter("x", [RPC, D], FP, isOutput=False)
    xt_in = nc.declare_dram_parameter("xt", [D, RPC], FPR, isOutput=False)
    wkv_in = nc.declare_dram_parameter("wkv", [D, 2 * H], FPR, isOutput=False)
    wqr_in = nc.declare_dram_parameter("wqr", [D, F], FPR, isOutput=False)
    w1_in = nc.declare_dram_parameter("w1", [D, H], BF, isOutput=False)
    w2_in = nc.declare_dram_parameter("w2", [H, D], BF, isOutput=False)
    out_ext = nc.declare_dram_parameter("out", [RPC, D], FP, isOutput=True)

    with tile.TileContext(nc) as tc:
        with (
            tc.tile_pool(name="const", bufs=1) as const_pool,
            tc.tile_pool(name="wpool", bufs=1) as wpool,
            tc.tile_pool(name="store", bufs=1) as store_pool,
            tc.tile_pool(name="dram", bufs=1, space="DRAM") as dram_pool,
        ):
            ident = const_pool.tile([128, 128], BF)
            masks.make_identity(nc, ident[:])
            eps_t = const_pool.tile([128, 1], FP)
            nc.vector.memset(eps_t[:], LN_EPS)
            qsh_t = const_pool.tile([128, 1], FP)
            nc.vector.memset(qsh_t[:], -QSHIFT)

            # ---- input DMA: k-side first (gates phase A1), halves split for
            # DMA-engine parallelism.
            xt_sb = store_pool.tile([128, 2, RPC], FPR, name="xt_sb")
            for c in (0, 1):
                for hlf in (0, 1):
                    nc.sync.dma_start(
                        out=xt_sb[:, c, hlf * 512 : (hlf + 1) * 512],
                        in_=xt_in[c * 128 : (c + 1) * 128,
                                  hlf * 512 : (hlf + 1) * 512],
                    )
            wkv = wpool.tile([128, 2, 2 * H], FPR, name="wkv_sb")
            for c in (0, 1):
                nc.sync.dma_start(
                    out=wkv[:, c, :], in_=wkv_in[c * 128 : (c + 1) * 128, :]
                )
            wqr = wpool.tile([128, 2, F], FPR, name="wqr_sb")
            for c in (0, 1):
                nc.sync.dma_start(
                    out=wqr[:, c, :], in_=wqr_in[c * 128 : (c + 1) * 128, :]
                )
            x_sb = store_pool.tile([128, TILES, D], FP, name="x_sb")
            for t in range(TILES):
                nc.sync.dma_start(
                    out=x_sb[:, t, :], in_=x_in[t * 128 : (t + 1) * 128, :]
                )
            w1 = wpool.tile([128, 2, H], BF, name="w1_sb")
            w2 = wpool.tile([128, 2, D], BF, name="w2_sb")
            for c in (0, 1):
                nc.sync.dma_start(
                    out=w1[:, c, :], in_=w1_in[c * 128 : (c + 1) * 128, :]
                )
                nc.sync.dma_start(
                    out=w2[:, c, :], in_=w2_in[c * 128 : (c + 1) * 128, :]
                )

            eqT_sb = store_pool.tile([128, 2, RPC], BF, name="eqT_sb")
            s_red = store_pool.tile([128, 2, SA], BF, name="s_red")
            s_bf = store_pool.tile([128, 2, SA], BF, name="s_bf")
            h_sb = store_pool.tile([128, TILES, D], BF, name="h_sb")

            # ---------------- Phase A1: k-side -> local S ---------------------
            with (
                tc.tile_pool(name="a_sb", bufs=3) as a_sb,
                tc.tile_pool(name="a_ps", bufs=2, space="PSUM") as a_ps,
                tc.tile_pool(name="s_ps", bufs=1, space="PSUM") as s_ps,
            ):
                s_psum = [
                    s_ps.tile([128, SA], FP, name=f"s_psum{c}") for c in (0, 1)
                ]
                ek_t = [None] * TILES
                va_t = [None] * TILES

                def a1_front(t):
                    # k cols 0:256 | v cols 256:512 in one PSUM bank, one
                    # accumulation group over the two D-chunks.
                    kv_ps = a_ps.tile([128, 2 * H], FP)
                    for c in (0, 1):
                        nc.tensor.matmul(
                            kv_ps[:],
                            xt_sb[:, c, t * 128 : (t + 1) * 128],
                            wkv[:, c, :],
                            start=c == 0,
                            stop=c == 1,
                        )
                    nmk = a_sb.tile([128, 1], FP)
                    nc.vector.tensor_reduce(
                        out=nmk[:], in_=kv_ps[:, 0:H], axis=AX.X, op=ALU.max,
                        negate=True,
                    )
                    ek = a_sb.tile([128, F], BF, name="ek_keep")
                    nc.scalar.activation(ek[:], kv_ps[:, 0:H], AF.Exp, bias=nmk[:])
                    va = a_sb.tile([128, SA], BF, name="va_keep")
                    nc.scalar.copy(va[:, 0:H], kv_ps[:, H : 2 * H])
                    nc.gpsimd.memset(va[:, H:SA], 1.0)
                    ek_t[t] = ek
                    va_t[t] = va

                def a1_smm(t):
                    for c in (0, 1):
                        nc.tensor.matmul(
                            s_psum[c][:],
                            ek_t[t][:, c * 128 : (c + 1) * 128],
                            va_t[t][:],
                            start=t == 0,
                            stop=t == TILES - 1,
                        )

                # one-tile skew: S matmuls never stall the tensor queue on exp
                for t in range(TILES):
                    a1_front(t)
                    if t >= 1:
                        a1_smm(t - 1)
                a1_smm(TILES - 1)

                for c in (0, 1):
                    nc.scalar.copy(s_bf[:, c, :], s_psum[c][:])

            # ---------------- AllReduce of S_aug (bf16) across 8 cores -------
            cc_in = dram_pool.tile([128, 2, SA], BF)
            cc_out = dram_pool.tile([128, 2, SA], BF, addr_space="Shared")
            nc.sync.dma_start(out=cc_in[:], in_=s_bf[:])
            nc.gpsimd.collective_compute(
                "AllReduce",
                ALU.add,
                replica_groups=[list(range(NC))],
                ins=[cc_in[:].opt()],
                outs=[cc_out[:].opt()],
            )
            nc.sync.dma_start(out=s_red[:], in_=cc_out[:])

            # ---------------- Phase A2: q-side transposed (under AllReduce) --
            # eqT = exp(Wqr^T x^T - 64) computed directly in [F, rows] layout:
            # no row-max (a constant shift cancels in num/denom), no
            # transposes, weights stationary with 512-wide moving operand.
            with tc.tile_pool(name="q_ps", bufs=2, space="PSUM") as q_ps:
                for fc in (0, 1):
                    for g in (0, 1):
                        qp_ps = q_ps.tile([128, 512], FP)
                        for c in (0, 1):
                            nc.tensor.matmul(
                                qp_ps[:],
                                wqr[:, c, fc * 128 : (fc + 1) * 128],
                                xt_sb[:, c, g * 512 : (g + 1) * 512],
                                start=c == 0,
                                stop=c == 1,
                            )
                        nc.scalar.activation(
                            eqT_sb[:, fc, g * 512 : (g + 1) * 512],
                            qp_ps[:],
                            AF.Exp,
                            bias=qsh_t[:],
                        )

            # ---------------- Phase B: numer, LN1, FFN, LN2 ------------------
            NQ = TILES // 4  # quads
            with (
                tc.tile_pool(name="b_sb", bufs=6) as b_sb,
                tc.tile_pool(name="p_num", bufs=2, space="PSUM") as p_num,
                tc.tile_pool(name="p_hT", bufs=2, space="PSUM") as p_hT,
                tc.tile_pool(name="p_ff1", bufs=2, space="PSUM") as p_ff1,
                tc.tile_pool(name="p_ff2", bufs=2, space="PSUM") as p_ff2,
            ):
                hT_q = [None] * NQ
                f1T_q = [None] * NQ

                def _ln(out_ap, in_ap):
                    # stats on vector; sqrt on scalar; apply on scalar via
                    # Identity(in*rstd + (-mu*rstd)).
                    stats = b_sb.tile([128, 6], FP)
                    aggr = b_sb.tile([128, 2], FP)
                    nc.vector.bn_stats(stats[:], in_ap)
                    nc.vector.bn_aggr(aggr[:], stats[:])
                    std = b_sb.tile([128, 1], FP)
                    nc.scalar.activation(std[:], aggr[:, 1:2], AF.Sqrt, bias=eps_t[:])
                    rstd = b_sb.tile([128, 1], FP)
                    nc.vector.reciprocal(rstd[:], std[:])
                    nmr = b_sb.tile([128, 1], FP)
                    nc.vector.tensor_scalar(
                        nmr[:], aggr[:, 0:1], rstd[:], -1.0, ALU.mult, ALU.mult
                    )
                    nc.scalar.activation(
                        out_ap, in_ap, AF.Identity, bias=nmr[:], scale=rstd[:]
                    )

                def stage1(t):
                    num_ps = p_num.tile([128, SA], FP, name="num_ps")
                    for c in (0, 1):
                        nc.tensor.matmul(
                            num_ps[:],
                            eqT_sb[:, c, t * 128 : (t + 1) * 128],
                            s_red[:, c, :],
                            start=c == 0,
                            stop=c == 1,
                        )
                    r = b_sb.tile([128, 1], FP)
                    nc.vector.reciprocal(r[:], num_ps[:, H : H + 1])
                    hin = b_sb.tile([128, D], BF)
                    nc.vector.scalar_tensor_tensor(
                        out=hin[:],
                        in0=num_ps[:, 0:D],
                        scalar=r[:],
                        in1=x_sb[:, t, :],
                        op0=ALU.mult,
                        op1=ALU.add,
                    )
                    _ln(h_sb[:, t, :], hin[:])

                def s2_transpose(qd):
                    hT2 = b_sb.tile([128, 2, 512], BF, name="hT2")
                    for dc in (0, 1):
                        ps = p_hT.tile([128, 512], BF, name="hT_ps")
                        for j in range(4):
                            nc.tensor.transpose(
                                ps[:, j * 128 : (j + 1) * 128],
                                h_sb[:, 4 * qd + j, dc * 128 : (dc + 1) * 128],
                                ident[:],
                            )
                        nc.scalar.copy(hT2[:, dc, :], ps[:])
                    hT_q[qd] = hT2

                def s2_ffn1(qd):
                    f1T = b_sb.tile([128, 2, 512], BF, name="f1T2")
                    for hc in (0, 1):
                        pre = p_ff1.tile([128, 512], FP, name="pre1T")
                        for dc in (0, 1):
                            nc.tensor.matmul(
                                pre[:],
                                w1[:, dc, hc * 128 : (hc + 1) * 128],
                                hT_q[qd][:, dc, :],
                                start=dc == 0,
                                stop=dc == 1,
                            )
                        nc.scalar.activation(f1T[:, hc, :], pre[:], AF.Relu)
                    f1T_q[qd] = f1T

                def stage3(t):
                    qd, j = t // 4, t % 4
                    ff2_ps = p_ff2.tile([128, D], FP, name="ff2_ps")
                    for hc in (0, 1):
                        nc.tensor.matmul(
                            ff2_ps[:],
                            f1T_q[qd][:, hc, j * 128 : (j + 1) * 128],
                            w2[:, hc, :],
                            start=hc == 0,
                            stop=hc == 1,
                        )
                    h2 = b_sb.tile([128, D], BF)
                    nc.vector.scalar_tensor_tensor(
                        out=h2[:], in0=ff2_ps[:], scalar=0.0,
                        in1=h_sb[:, t, :], op0=ALU.bypass, op1=ALU.add,
                    )
                    outt = b_sb.tile([128, D], FP)
                    _ln(outt[:], h2[:])
                    nc.sync.dma_start(
                        out=out_ext[t * 128 : (t + 1) * 128, :], in_=outt[:]
                    )

                stage1(0); stage1(1); stage1(2); stage1(3)
                s2_transpose(0)
                s2_ffn1(0)
                stage1(4); stage1(5); stage1(6); stage1(7)
                stage3(0); stage3(1)
                s2_transpose(1)
                stage3(2); stage3(3)
                s2_ffn1(1)
                stage3(4); stage3(5); stage3(6); stage3(7)

    nc.finalize()
    return nc


_NC_CACHE = {}


def _get_nc():
    if "nc" not in _NC_CACHE:
        _NC_CACHE["nc"] = _build_kernel()
    return _NC_CACHE["nc"]


def _run(inputs, trace=False, **kw):
    bf16 = ml_dtypes.bfloat16
    x = np.ascontiguousarray(inputs["x"], dtype=np.float32)
    R = inputs["R"].astype(np.float64)
    wqr = (inputs["Wq"].astype(np.float64) @ R).astype(np.float32)
    wkr = (inputs["Wk"].astype(np.float64) @ R).astype(np.float32)
    wvo = (
        inputs["Wv"].astype(np.float64) @ inputs["Wo"].astype(np.float64)
    ).astype(np.float32)
    wkv = np.ascontiguousarray(np.concatenate([wkr, wvo], axis=1))
    shared = {
        "wkv": wkv,
        "wqr": np.ascontiguousarray(wqr),
        "w1": np.ascontiguousarray(inputs["W1"]).astype(bf16),
        "w2": np.ascontiguousarray(inputs["W2"]).astype(bf16),
    }
    in_maps = []
    for c in range(NC):
        slab = np.ascontiguousarray(x[c * RPC : (c + 1) * RPC])
        in_maps.append(
            {
                "x": slab,
                "xt": np.ascontiguousarray(slab.T),
                **shared,
            }
        )
    nc = _get_nc()
    res = run_bass_kernel_spmd(nc, in_maps, list(range(NC)), trace=trace, **kw)
    out = np.concatenate([res.results[c]["out"] for c in range(NC)], axis=0)
    return out.astype(np.float32), res


def kernel(**inputs) -> np.ndarray:
    out, _ = _run(inputs)
    return out
